# revision 14
# baseline (speedup 1.0000x reference)
"""Trainium2 Bass kernel for LocalWindowAttention (B=8, C=256, H=W=64, r=32).

Strategy: data-parallel over batch across 8 NeuronCores (one batch element
per core, zero collectives).  Per core everything is computed in a
"transposed" layout so that the softmax denominator (a sum over the
contraction axis m, which lives on SBUF partitions) can be computed with
cheap TensorE ones-matmuls instead of the slow VectorE.

v3: the per-execute dispatch cost through the PJRT path scales hard with
the number of I/O buffers (~2ms/operand) and dma_start count (~20us each),
so the program has exactly ONE input and ONE output: the weights/biases/
constants are packed host-side into 592 extra columns of the x operand
("xext", (256, 4688)), and the kernel copies those columns through to the
same place in the output so chained executions (output fed back as input)
are self-sustaining.

  qT  (r=32 x N)   = (wq*scale) @ x + bq*scale   replicated to 4 row groups
  kT  (r=32 x N)   = wk @ x + bk                 (full-width SBUF DMAs)
  vT  (m x C)      = x^T @ wv^T + bv             (ones-matmul bias trick)
  sT  (m x n)      = kT_slice^T @ qT             row-tiled (K=32 packing)
  E = exp(sT)      on ScalarE, PSUM -> SBUF (rotating pool)
  colsum           = ones^T @ E   per m-chunk accumulated into one bank
  out_u (C x n)    = vT^T @ E     accumulated over m-chunks
  out = x + gamma * out_u / colsum, staged in SBUF, 2 final stores
"""

import hashlib
import numpy as np
from contextlib import ExitStack

import concourse.bass as bass
import concourse.tile as tile
from concourse import bacc, mybir, bass_utils

F32 = mybir.dt.float32
BF16 = mybir.dt.bfloat16
FP8 = mybir.dt.float8e4
AF = mybir.ActivationFunctionType
ALU = mybir.AluOpType
F32R = mybir.dt.float32r


B, C, HH, WW = 8, 256, 64, 64
N = HH * WW            # 4096 tokens
R = 32                 # low-rank q/k dim
NCORES = 8
NB = 512               # n-block (free dim per matmul)
NNB = N // NB          # 8
MC = 128               # m-chunk (contraction tile)
NMC = N // MC          # 32

# weight-blob column layout (128 partitions x _WBLOB f32), carried in the
# trailing _XTRA columns of the xext operand (rows 0:128 hold blob cols
# [0:_XTRA), rows 128:256 hold blob cols [_XTRA:2*_XTRA))
_WQ0 = 0               # [0:64)    wqt*scale, k-chunk-major
_WK0 = 64              # [64:128)  wkt
_WV0 = 128             # [128:640) wvt, k-chunk-major (2 x 256)
_BQ = 640              # [640]     bq*scale tiled 4x
_BK = 641              # [641]     bk tiled 4x
_BVC = 642             # [642:644) bv as two 128-row columns
_WBLOB = 656           # padded
_XTRA = _WBLOB // 2    # 592 extra columns on each 128-row half
NX = N + _XTRA         # 4688 columns of the xext operand

_cache = {}


def _make_blob(wq, bq, wk, bk, wv, bv):
    scale = float(R) ** -0.5
    blob = np.zeros((128, _WBLOB), np.float32)
    for k in range(2):
        blob[:, _WQ0 + k * 32:_WQ0 + (k + 1) * 32] = (wq[:, k * 128:(k + 1) * 128] * scale).T
        blob[:, _WK0 + k * 32:_WK0 + (k + 1) * 32] = wk[:, k * 128:(k + 1) * 128].T
        blob[:, _WV0 + k * 256:_WV0 + (k + 1) * 256] = wv[:, k * 128:(k + 1) * 128].T
    blob[:, _BQ] = np.tile(bq * scale, 4)
    blob[:, _BK] = np.tile(bk, 4)
    blob[:, _BVC] = bv[0:128]
    blob[:, _BVC + 1] = bv[128:256]
    return blob


def _build_program(gamma):
    nc = bacc.Bacc("TRN2", debug=False, enable_asserts=True, num_devices=NCORES)
    x_d = nc.dram_tensor("x", (C, NX), BF16, kind="ExternalInput").ap()
    out_d = nc.dram_tensor("out", (C, NX), BF16, kind="ExternalOutput").ap()

    with tile.TileContext(nc) as tc, ExitStack() as ctx, \
         nc.allow_low_precision(reason="fp32r matmul streaming mode"):
        consts = ctx.enter_context(tc.tile_pool(name="consts", bufs=1))
        xpool = ctx.enter_context(tc.tile_pool(name="xp", bufs=1))
        qkpool = ctx.enter_context(tc.tile_pool(name="qk", bufs=1))
        vpool = ctx.enter_context(tc.tile_pool(name="vp", bufs=1))
        exppool = ctx.enter_context(tc.tile_pool(name="ep", bufs=3))
        opool = ctx.enter_context(tc.tile_pool(name="op", bufs=1))
        misc = ctx.enter_context(tc.tile_pool(name="misc", bufs=1))

        # weight blob rides in the tail columns of xext
        blob_sb = consts.tile([128, _WBLOB], BF16, tag="blob")
        nc.sync.dma_start(
            blob_sb[:].rearrange("p (k c) -> p k c", k=2),
            x_d[:, N:NX].rearrange("(k p) c -> p k c", p=128))
        # pass the weight columns through to the output so chained
        # executions (out fed back as x) keep working
        nc.sync.dma_start(out_d[:, N:NX], x_d[:, N:NX])

        def bw(c0, c1):
            return blob_sb[:, c0:c1]

        xt = []
        for k in range(2):
            t = xpool.tile([128, N], BF16, tag=f"xt{k}", name=f"xt{k}")
            nc.sync.dma_start(t[:], x_d[k * 128:(k + 1) * 128, 0:N])
            xt.append(t)
        x8 = xpool.tile([128, 2, N], FP8, tag="x8")
        for k in range(2):
            nc.vector.tensor_copy(x8[:, k, :], xt[k][:])

        # exact f32 ones row for the softmax-denominator broadcast
        onesr_f = consts.tile([1, 128], F32, tag="onesrf")
        nc.vector.memset(onesr_f[:], 1.0)
        # fp8 ones pair (stride-16 padded) for the DoubleRow colsum
        ones8 = consts.tile([128, 2, 16], FP8, tag="ones8")
        nc.vector.memset(ones8[:], 1.0)
        # f32 copies of the bias columns (tensor_scalar needs f32 scalars)
        bias_f = consts.tile([128, 4], F32, tag="biasf")
        nc.vector.tensor_copy(bias_f[:], blob_sb[:, _BQ:_BQ + 4])
        # fp8 copies of x and the projection weights for DoubleRow matmuls
        wv8 = consts.tile([128, 2, C], FP8, tag="wv8")
        nc.vector.tensor_copy(
            wv8[:], blob_sb[:, _WV0:_WV0 + 2 * C].rearrange(
                "p (two c) -> p two c", two=2))
        wqk8 = consts.tile([128, 128], FP8, tag="wqk8")
        nc.vector.tensor_copy(wqk8[:], blob_sb[:, 0:128])

        # ---- persistent activations ----
        qt_sb = qkpool.tile([128, N], BF16, tag="qt")     # q^T replicated 4x
        kt_sb = qkpool.tile([128, N], BF16, tag="kt")     # k^T replicated 4x
        vt_sb = vpool.tile([128, NMC, C], FP8, tag="vt")  # v^T, chunk j at [:, j, :]
        out_sb = [opool.tile([128, N], BF16, tag=f"out{ch}", name=f"out_sb{ch}")
                  for ch in range(2)]

        # ================= phase 1: q/k/v projections =================
        DR = mybir.MatmulPerfMode.DoubleRow
        wq8_ap = wqk8[:, 0:64].rearrange("p (two m) -> p two m", two=2)
        wk8_ap = wqk8[:, 64:128].rearrange("p (two m) -> p two m", two=2)
        with tc.tile_pool(name="pp", bufs=2, space=bass.MemorySpace.PSUM) as pp, \
             tc.tile_pool(name="pvp", bufs=2, space=bass.MemorySpace.PSUM) as pvp:
            for i in range(8):
                cols = slice(i * NB, (i + 1) * NB)
                for (w8, dst, bcol) in ((wq8_ap, qt_sb, 0), (wk8_ap, kt_sb, 1)):
                    pt = pp.tile([128, NB], F32, tag="pp", name=f"pp_{i}_{bcol}")
                    nc.tensor.matmul(pt[0:32, :], w8, x8[:, :, cols],
                                     start=True, stop=True, perf_mode=DR)
                    nc.vector.tensor_scalar_add(
                        dst[0:32, cols], pt[0:32, :],
                        bias_f[0:32, bcol:bcol + 1])
                # vT for the 4 m-chunks of this column block (bias folded
                # into the epilogue: sum((v+b)E)/sumE == sum(vE)/sumE + b)
                for jj in range(4):
                    j = 4 * i + jj
                    pv = pvp.tile([128, C], F32, tag="pv", name=f"pv_{j}")
                    nc.tensor.matmul(pv[:], x8[:, :, j * MC:(j + 1) * MC],
                                     wv8[:], start=True, stop=True, perf_mode=DR)
                    nc.vector.tensor_copy(vt_sb[:, j, :], pv[:])
            # replicate q^T/k^T rows 0:32 -> 32:64, 64:96, 96:128 for the
            # row-tiled s-matmuls (one full-width DMA per replica)
            for dst in (qt_sb, kt_sb):
                for g in range(1, 4):
                    nc.sync.dma_start(dst[32 * g:32 * (g + 1), :], dst[0:32, :])

        # ================= phase 2: attention =================
        with tc.tile_pool(name="ps", bufs=1, space=bass.MemorySpace.PSUM) as psp, \
             tc.tile_pool(name="po", bufs=2, space=bass.MemorySpace.PSUM) as pop, \
             tc.tile_pool(name="pc", bufs=2, space=bass.MemorySpace.PSUM) as pcp:
            for nb in range(NNB):
                ncols = slice(nb * NB, (nb + 1) * NB)
                po_t = [pop.tile([128, NB], F32, tag="po", name=f"po_{nb}_{i}")
                        for i in range(2)]
                pc_t = pcp.tile([128, NB], F32, tag="pc", name=f"pc_{nb}")
                exp_tiles = {}

                def consume(tt, po_t=po_t, pc_t=pc_t, exp_tiles=exp_tiles):
                    # two DoubleRow chunk-pairs per exp tile (chunks 4tt..4tt+3)
                    et = exp_tiles.pop(tt)
                    for p in range(2):
                        u = 2 * tt + p
                        nc.tensor.matmul(
                            pc_t[0:1, :],
                            ones8[:, :, 0:1],
                            et[:, p, :, :],
                            start=(u == 0), stop=(u == 15),
                            perf_mode=mybir.MatmulPerfMode.DoubleRow)
                    for p in range(2):
                        u = 2 * tt + p
                        for ch in range(2):
                            nc.tensor.matmul(
                                po_t[ch][:],
                                vt_sb[:, 2 * u:2 * u + 2, ch * 128:(ch + 1) * 128],
                                et[:, p, :, :],
                                start=(u == 0), stop=(u == 15),
                                perf_mode=mybir.MatmulPerfMode.DoubleRow)

                for t in range(8):
                    ps_t = psp.tile([128, 4 * NB], F32, tag="ps", name=f"ps_{nb}_{t}")
                    for g in range(4):
                        j = 4 * t + g
                        nc.tensor.matmul(
                            ps_t[:, g * NB:(g + 1) * NB],
                            kt_sb[32 * g:32 * (g + 1), j * MC:(j + 1) * MC],
                            qt_sb[32 * g:32 * (g + 1), ncols],
                            start=True, stop=True,
                            tile_position=(32 * g, 0))
                    et = exppool.tile([128, 2, 2, NB], FP8, tag="exp",
                                      name=f"exp_{nb}_{t}")
                    nc.scalar.activation(
                        et[:].rearrange("p a two n -> p (a two n)"), ps_t[:], AF.Exp)
                    exp_tiles[t] = et
                    if t >= 1:
                        consume(t - 1)
                consume(7)

                # ---- softmax denominator -> reciprocal -> epilogue ----
                recip = misc.tile([1, NB], F32, tag="recip", bufs=2, name=f"recip_{nb}")
                nc.vector.reciprocal(recip[:], pc_t[0:1, :])
                pb_t = pcp.tile([128, NB], F32, tag="pc", name=f"pb_{nb}")
                nc.tensor.matmul(pb_t[:], onesr_f[:],
                                 recip[:], start=True, stop=True)
                bc_sb = misc.tile([128, NB], F32, tag="bc", name=f"bc_{nb}")
                nc.vector.tensor_copy(bc_sb[:], pb_t[:])
                for ch in range(2):
                    tmp = misc.tile([128, NB], F32, tag="tmp", bufs=2,
                                    name=f"tmp_{nb}_{ch}")
                    nc.vector.tensor_mul(tmp[:], po_t[ch][:], bc_sb[:])
                    tmp2 = misc.tile([128, NB], F32, tag="tmp2", bufs=2,
                                     name=f"tmp2_{nb}_{ch}")
                    nc.vector.tensor_scalar(
                        tmp2[:], tmp[:], bias_f[:, 2 + ch:3 + ch], gamma,
                        ALU.add, ALU.mult)
                    nc.vector.tensor_add(
                        out_sb[ch][:, ncols], tmp2[:], xt[ch][:, ncols])

            for ch in range(2):
                nc.sync.dma_start(out_d[ch * 128:(ch + 1) * 128, 0:N], out_sb[ch][:])

    nc.compile()
    return nc


def _get_nc(inputs):
    gamma = float(np.asarray(inputs["gamma"]).reshape(-1)[0])
    key = repr(gamma)
    if key not in _cache:
        _cache.clear()
        _cache[key] = _build_program(gamma)
    return _cache[key]


def _make_in_maps(inputs):
    import ml_dtypes
    x = np.asarray(inputs["x"], dtype=np.float32)
    wq = np.ascontiguousarray(np.asarray(inputs["wq"], dtype=np.float32))
    bq = np.ascontiguousarray(np.asarray(inputs["bq"], dtype=np.float32))
    wk = np.ascontiguousarray(np.asarray(inputs["wk"], dtype=np.float32))
    bk = np.ascontiguousarray(np.asarray(inputs["bk"], dtype=np.float32))
    wv = np.ascontiguousarray(np.asarray(inputs["wv"], dtype=np.float32))
    bv = np.ascontiguousarray(np.asarray(inputs["bv"], dtype=np.float32))
    blob = _make_blob(wq, bq, wk, bk, wv, bv)
    tail = np.concatenate([blob[:, :_XTRA], blob[:, _XTRA:]], axis=0)
    tail = tail.astype(ml_dtypes.bfloat16)
    in_maps = []
    for b in range(B):
        xe = np.empty((C, NX), ml_dtypes.bfloat16)
        xe[:, 0:N] = x[b].reshape(C, N).astype(ml_dtypes.bfloat16)
        xe[:, N:] = tail
        in_maps.append({"x": xe})
    return in_maps


def kernel(**inputs) -> np.ndarray:
    nc = _get_nc(inputs)
    in_maps = _make_in_maps(inputs)
    res = bass_utils.run_bass_kernel_spmd(nc, in_maps, core_ids=list(range(NCORES)))
    out = np.stack([np.asarray(res.results[b]["out"][:, 0:N], dtype=np.float32)
                    .reshape(C, HH, WW) for b in range(B)])
    return out


# revision 15
# speedup vs baseline: 3.8978x; 3.8978x over previous
"""Trainium2 Bass kernel for LocalWindowAttention (B=8, C=256, H=W=64, r=32).

Strategy: data-parallel over batch across 8 NeuronCores (one batch element
per core, zero collectives).  Per core everything is computed in a
"transposed" layout so that the softmax denominator (a sum over the
contraction axis m, which lives on SBUF partitions) can be computed with
cheap TensorE ones-matmuls instead of the slow VectorE.

v3: the per-execute dispatch cost through the PJRT path scales hard with
the number of I/O buffers (~2ms/operand) and dma_start count (~20us each),
so the program has exactly ONE input and ONE output: the weights/biases/
constants are packed host-side into 592 extra columns of the x operand
("xext", (256, 4688)), and the kernel copies those columns through to the
same place in the output so chained executions (output fed back as input)
are self-sustaining.

  qT  (r=32 x N)   = (wq*scale) @ x + bq*scale   replicated to 4 row groups
  kT  (r=32 x N)   = wk @ x + bk                 (full-width SBUF DMAs)
  vT  (m x C)      = x^T @ wv^T + bv             (ones-matmul bias trick)
  sT  (m x n)      = kT_slice^T @ qT             row-tiled (K=32 packing)
  E = exp(sT)      on ScalarE, PSUM -> SBUF (rotating pool)
  colsum           = ones^T @ E   per m-chunk accumulated into one bank
  out_u (C x n)    = vT^T @ E     accumulated over m-chunks
  out = x + gamma * out_u / colsum, staged in SBUF, 2 final stores
"""

import hashlib
import numpy as np
from contextlib import ExitStack

import concourse.bass as bass
import concourse.tile as tile
from concourse import bacc, mybir, bass_utils

F32 = mybir.dt.float32
BF16 = mybir.dt.bfloat16
FP8 = mybir.dt.float8e4
AF = mybir.ActivationFunctionType
ALU = mybir.AluOpType
F32R = mybir.dt.float32r


B, C, HH, WW = 8, 256, 64, 64
N = HH * WW            # 4096 tokens
R = 32                 # low-rank q/k dim
NCORES = 8
NB = 512               # n-block (free dim per matmul)
NNB = N // NB          # 8
MC = 128               # m-chunk (contraction tile)
NMC = N // MC          # 32

# weight-blob column layout (128 partitions x _WBLOB f32), carried in the
# trailing _XTRA columns of the xext operand (rows 0:128 hold blob cols
# [0:_XTRA), rows 128:256 hold blob cols [_XTRA:2*_XTRA))
_WQ0 = 0               # [0:64)    wqt*scale, k-chunk-major
_WK0 = 64              # [64:128)  wkt
_WV0 = 128             # [128:640) wvt, k-chunk-major (2 x 256)
_BQ = 640              # [640]     bq*scale tiled 4x
_BK = 641              # [641]     bk tiled 4x
_BVC = 642             # [642:644) bv as two 128-row columns
_WBLOB = 656           # padded
_XTRA = _WBLOB // 2    # 592 extra columns on each 128-row half
NX = N + _XTRA         # 4688 columns of the xext operand

_cache = {}


def _make_blob(wq, bq, wk, bk, wv, bv):
    scale = float(R) ** -0.5
    blob = np.zeros((128, _WBLOB), np.float32)
    for k in range(2):
        blob[:, _WQ0 + k * 32:_WQ0 + (k + 1) * 32] = (wq[:, k * 128:(k + 1) * 128] * scale).T
        blob[:, _WK0 + k * 32:_WK0 + (k + 1) * 32] = wk[:, k * 128:(k + 1) * 128].T
        blob[:, _WV0 + k * 256:_WV0 + (k + 1) * 256] = wv[:, k * 128:(k + 1) * 128].T
    blob[:, _BQ] = np.tile(bq * scale, 4)
    blob[:, _BK] = np.tile(bk, 4)
    blob[:, _BVC] = bv[0:128]
    blob[:, _BVC + 1] = bv[128:256]
    return blob


def _build_program(gamma):
    nc = bacc.Bacc("TRN2", debug=False, enable_asserts=True, num_devices=NCORES)
    x_d = nc.dram_tensor("x", (C, NX), BF16, kind="ExternalInput").ap()
    out_d = nc.dram_tensor("out", (C, NX), BF16, kind="ExternalOutput").ap()

    with tile.TileContext(nc) as tc, ExitStack() as ctx, \
         nc.allow_low_precision(reason="fp32r matmul streaming mode"):
        consts = ctx.enter_context(tc.tile_pool(name="consts", bufs=1))
        xpool = ctx.enter_context(tc.tile_pool(name="xp", bufs=1))
        qkpool = ctx.enter_context(tc.tile_pool(name="qk", bufs=1))
        vpool = ctx.enter_context(tc.tile_pool(name="vp", bufs=1))
        exppool = ctx.enter_context(tc.tile_pool(name="ep", bufs=3))
        opool = ctx.enter_context(tc.tile_pool(name="op", bufs=1))
        misc = ctx.enter_context(tc.tile_pool(name="misc", bufs=1))

        # weight blob rides in the tail columns of xext
        blob_sb = consts.tile([128, _WBLOB], BF16, tag="blob")
        nc.sync.dma_start(
            blob_sb[:].rearrange("p (k c) -> p k c", k=2),
            x_d[:, N:NX].rearrange("(k p) c -> p k c", p=128))
        # pass the weight columns through to the output so chained
        # executions (out fed back as x) keep working
        nc.sync.dma_start(out_d[:, N:NX], x_d[:, N:NX])

        def bw(c0, c1):
            return blob_sb[:, c0:c1]

        xt = []
        for k in range(2):
            t = xpool.tile([128, N], BF16, tag=f"xt{k}", name=f"xt{k}")
            nc.sync.dma_start(t[:], x_d[k * 128:(k + 1) * 128, 0:N])
            xt.append(t)
        x8 = xpool.tile([128, 2, N], FP8, tag="x8")
        for k in range(2):
            nc.vector.tensor_copy(x8[:, k, :], xt[k][:])

        # exact f32 ones row for the softmax-denominator broadcast
        onesr_f = consts.tile([1, 128], F32, tag="onesrf")
        nc.vector.memset(onesr_f[:], 1.0)
        # fp8 ones pair (stride-16 padded) for the DoubleRow colsum
        ones8 = consts.tile([128, 2, 16], FP8, tag="ones8")
        nc.vector.memset(ones8[:], 1.0)
        # f32 copies of the bias columns (tensor_scalar needs f32 scalars)
        bias_f = consts.tile([128, 4], F32, tag="biasf")
        nc.vector.tensor_copy(bias_f[:], blob_sb[:, _BQ:_BQ + 4])
        # fp8 copies of x and the projection weights for DoubleRow matmuls
        wv8 = consts.tile([128, 2, C], FP8, tag="wv8")
        nc.vector.tensor_copy(
            wv8[:], blob_sb[:, _WV0:_WV0 + 2 * C].rearrange(
                "p (two c) -> p two c", two=2))
        wqk8 = consts.tile([128, 128], FP8, tag="wqk8")
        nc.vector.tensor_copy(wqk8[:], blob_sb[:, 0:128])

        # ---- persistent activations ----
        qt_sb = qkpool.tile([128, N], BF16, tag="qt")     # q^T replicated 4x
        kt_sb = qkpool.tile([128, N], BF16, tag="kt")     # k^T replicated 4x
        vt_sb = vpool.tile([128, NMC, C], FP8, tag="vt")  # v^T, chunk j at [:, j, :]
        out_sb = [opool.tile([128, N], BF16, tag=f"out{ch}", name=f"out_sb{ch}")
                  for ch in range(2)]

        # ================= phase 1: q/k/v projections =================
        DR = mybir.MatmulPerfMode.DoubleRow
        wq8_ap = wqk8[:, 0:64].rearrange("p (two m) -> p two m", two=2)
        wk8_ap = wqk8[:, 64:128].rearrange("p (two m) -> p two m", two=2)
        with tc.tile_pool(name="pp", bufs=2, space=bass.MemorySpace.PSUM) as pp, \
             tc.tile_pool(name="pvp", bufs=2, space=bass.MemorySpace.PSUM) as pvp:
            for i in range(8):
                cols = slice(i * NB, (i + 1) * NB)
                for (w8, dst, bcol) in ((wq8_ap, qt_sb, 0), (wk8_ap, kt_sb, 1)):
                    pt = pp.tile([128, NB], F32, tag="pp", name=f"pp_{i}_{bcol}")
                    nc.tensor.matmul(pt[0:32, :], w8, x8[:, :, cols],
                                     start=True, stop=True, perf_mode=DR)
                    nc.vector.tensor_scalar_add(
                        dst[0:32, cols], pt[0:32, :],
                        bias_f[0:32, bcol:bcol + 1])
                # vT for the 4 m-chunks of this column block (bias folded
                # into the epilogue: sum((v+b)E)/sumE == sum(vE)/sumE + b)
                for jj in range(4):
                    j = 4 * i + jj
                    pv = pvp.tile([128, C], F32, tag="pv", name=f"pv_{j}")
                    nc.tensor.matmul(pv[:], x8[:, :, j * MC:(j + 1) * MC],
                                     wv8[:], start=True, stop=True, perf_mode=DR)
                    nc.vector.tensor_copy(vt_sb[:, j, :], pv[:])
            # replicate q^T/k^T rows 0:32 -> 32:64, 64:96, 96:128 for the
            # row-tiled s-matmuls (one full-width DMA per replica)
            for dst in (qt_sb, kt_sb):
                for g in range(1, 4):
                    nc.sync.dma_start(dst[32 * g:32 * (g + 1), :], dst[0:32, :])

        # ================= phase 2: attention =================
        with tc.tile_pool(name="ps", bufs=2, space=bass.MemorySpace.PSUM) as psp, \
             tc.tile_pool(name="po", bufs=2, space=bass.MemorySpace.PSUM) as pop, \
             tc.tile_pool(name="pc", bufs=2, space=bass.MemorySpace.PSUM) as pcp:
            for nb in range(NNB):
                ncols = slice(nb * NB, (nb + 1) * NB)
                po_t = [pop.tile([128, NB], F32, tag="po", name=f"po_{nb}_{i}")
                        for i in range(2)]
                pc_t = pcp.tile([128, NB], F32, tag="pc", name=f"pc_{nb}")
                exp_tiles = {}

                def consume(tt, po_t=po_t, pc_t=pc_t, exp_tiles=exp_tiles):
                    et = exp_tiles.pop(tt)
                    # DoubleRow fp8 matmuls: each handles the chunk pair
                    # (2*tt, 2*tt+1) in one instruction (K=256 virtual).
                    # colsum first so the denominator tail clears early.
                    nc.tensor.matmul(
                        pc_t[0:1, :],
                        ones8[:, :, 0:1],
                        et[:],
                        start=(tt == 0), stop=(tt == 15),
                        perf_mode=mybir.MatmulPerfMode.DoubleRow)
                    for ch in range(2):
                        nc.tensor.matmul(
                            po_t[ch][:],
                            vt_sb[:, 2 * tt:2 * tt + 2, ch * 128:(ch + 1) * 128],
                            et[:],
                            start=(tt == 0), stop=(tt == 15),
                            perf_mode=mybir.MatmulPerfMode.DoubleRow)

                for t in range(16):
                    ps_t = psp.tile([128, 2 * NB], F32, tag="ps", name=f"ps_{nb}_{t}")
                    for g2 in range(2):
                        j = 2 * t + g2
                        gm = j % 4
                        nc.tensor.matmul(
                            ps_t[:, g2 * NB:(g2 + 1) * NB],
                            kt_sb[32 * gm:32 * (gm + 1), j * MC:(j + 1) * MC],
                            qt_sb[32 * gm:32 * (gm + 1), ncols],
                            start=True, stop=True,
                            tile_position=(32 * gm, 0))
                    et = exppool.tile([128, 2, NB], FP8, tag="exp",
                                      name=f"exp_{nb}_{t}")
                    nc.scalar.activation(
                        et[:].rearrange("p two n -> p (two n)"), ps_t[:], AF.Exp)
                    exp_tiles[t] = et
                    if t >= 1:
                        consume(t - 1)
                consume(15)

                # ---- softmax denominator -> reciprocal -> epilogue ----
                recip = misc.tile([1, NB], F32, tag="recip", bufs=2, name=f"recip_{nb}")
                nc.vector.reciprocal(recip[:], pc_t[0:1, :])
                pb_t = pcp.tile([128, NB], F32, tag="pc", name=f"pb_{nb}")
                nc.tensor.matmul(pb_t[:], onesr_f[:],
                                 recip[:], start=True, stop=True)
                bc_sb = misc.tile([128, NB], F32, tag="bc", name=f"bc_{nb}")
                nc.vector.tensor_copy(bc_sb[:], pb_t[:])
                for ch in range(2):
                    tmp = misc.tile([128, NB], F32, tag="tmp", bufs=2,
                                    name=f"tmp_{nb}_{ch}")
                    nc.vector.tensor_mul(tmp[:], po_t[ch][:], bc_sb[:])
                    tmp2 = misc.tile([128, NB], F32, tag="tmp2", bufs=2,
                                     name=f"tmp2_{nb}_{ch}")
                    nc.vector.tensor_scalar(
                        tmp2[:], tmp[:], bias_f[:, 2 + ch:3 + ch], gamma,
                        ALU.add, ALU.mult)
                    nc.vector.tensor_add(
                        out_sb[ch][:, ncols], tmp2[:], xt[ch][:, ncols])

            for ch in range(2):
                nc.sync.dma_start(out_d[ch * 128:(ch + 1) * 128, 0:N], out_sb[ch][:])

    nc.compile()
    return nc


def _get_nc(inputs):
    gamma = float(np.asarray(inputs["gamma"]).reshape(-1)[0])
    key = repr(gamma)
    if key not in _cache:
        _cache.clear()
        _cache[key] = _build_program(gamma)
    return _cache[key]


def _make_in_maps(inputs):
    import ml_dtypes
    x = np.asarray(inputs["x"], dtype=np.float32)
    wq = np.ascontiguousarray(np.asarray(inputs["wq"], dtype=np.float32))
    bq = np.ascontiguousarray(np.asarray(inputs["bq"], dtype=np.float32))
    wk = np.ascontiguousarray(np.asarray(inputs["wk"], dtype=np.float32))
    bk = np.ascontiguousarray(np.asarray(inputs["bk"], dtype=np.float32))
    wv = np.ascontiguousarray(np.asarray(inputs["wv"], dtype=np.float32))
    bv = np.ascontiguousarray(np.asarray(inputs["bv"], dtype=np.float32))
    blob = _make_blob(wq, bq, wk, bk, wv, bv)
    tail = np.concatenate([blob[:, :_XTRA], blob[:, _XTRA:]], axis=0)
    tail = tail.astype(ml_dtypes.bfloat16)
    in_maps = []
    for b in range(B):
        xe = np.empty((C, NX), ml_dtypes.bfloat16)
        xe[:, 0:N] = x[b].reshape(C, N).astype(ml_dtypes.bfloat16)
        xe[:, N:] = tail
        in_maps.append({"x": xe})
    return in_maps


def kernel(**inputs) -> np.ndarray:
    nc = _get_nc(inputs)
    in_maps = _make_in_maps(inputs)
    res = bass_utils.run_bass_kernel_spmd(nc, in_maps, core_ids=list(range(NCORES)))
    out = np.stack([np.asarray(res.results[b]["out"][:, 0:N], dtype=np.float32)
                    .reshape(C, HH, WW) for b in range(B)])
    return out
